# revision 33
# baseline (speedup 1.0000x reference)
"""MoE down-projection (grouped GEMM + topk combine) on 8 Trainium2 cores.

Strategy: expert-parallel. Each of the 8 cores owns E/8 = 16 experts and
receives (a) its experts' weight slabs and (b) the x rows routed to those
experts, gathered+gate-scaled+transposed on host, padded per expert to a
fixed capacity C. The device kernel is a block-diagonal grouped GEMM:
for each expert slot b: y[b] = xT[:, b*C:(b+1)*C].T @ w[b].
Weights stream through the PE as the moving operand (full rate); the few
x rows per expert are the stationary operand. Host scatter-adds the row
results back into the [T, H] output (each token has exactly K=2 rows).

The kernel is HBM-DMA bound, so w is quantized to fp8 e3m4 (4 mantissa
bits) with a per-expert scale whose inverse is folded into the host-side
x rows (each row feeds exactly one expert) — zero device-side descaling,
half the weight bytes of bf16, measured rel err 1.35e-2. x stays bf16
(mixed-dtype matmul), y returns as bf16. e4m3 would fail the 2e-2 gate.

Hardcoded problem shape (from the problem spec):
  x: [2048, 512] f32, w: [128, 512, 2048] f32,
  chosen_experts: [1024, 2] int, expert_weight: [1024, 2] f32 -> out [1024, 2048] f32
"""

import numpy as np

T = 1024
K_TOP = 2
E = 128
I_DIM = 512
H = 2048
N_CORES = 8
EPC = E // N_CORES  # experts per core = 16
P = 128             # partitions
I_CHUNKS = I_DIM // P       # 4
H_CHUNK = 512               # matmul moving free dim (fp32 PSUM bank)
H_CHUNKS = H // H_CHUNK     # 4

# matmul dtype: "float32" (exact, 4 cyc/row), "float32r" (1 cyc/row, reduced
# internal precision), "bfloat16" (1 cyc/row, half DMA traffic), "float8e3"
# (w in fp8 e3m4 with per-expert scale folded into the x rows, x in bf16,
# quarter DMA traffic for the weight stream)
DEFAULT_DTYPE = "float8e3"

# e3m4 absmax target: per-expert scale maps max|w_e| -> this value
# (e3m4 max normal = 15.5; stay just below to avoid overflow on rounding)
W8_TARGET = 15.0

_cache = {}


def _layout(C, dt_name):
    """PE column-group packing of the per-expert output. 4-byte matmuls
    reject tile_position col-tiling, so they use the flat layout."""
    G = max(1, P // C)
    if H_CHUNKS % G != 0 or dt_name not in ("bfloat16", "float8e3"):
        G = 1
    return G, H_CHUNKS // G, G * C


def _dtypes(dt_name):
    """(w dtype, x dtype, y dtype) names for a given matmul precision."""
    if dt_name == "float8e3":
        return "float8e3", "bfloat16", "bfloat16"
    if dt_name == "bfloat16":
        return "bfloat16", "bfloat16", "bfloat16"
    return dt_name, dt_name, "float32"


def _build(C: int, dt_name: str):
    import concourse.mybir as mybir
    import concourse.tile as tile
    from concourse import bacc

    wdt_name, xdt_name, ydt_name = _dtypes(dt_name)
    wdt = getattr(mybir.dt, wdt_name)
    xdt = getattr(mybir.dt, xdt_name)
    ydt = getattr(mybir.dt, ydt_name)
    # fp8 slabs are 8KB/partition, so all EPC fit in SBUF at once (one tag
    # rotating through EPC buffers): slab DMAs never wait on matmul progress
    wbufs = EPC if dt_name == "float8e3" else (6 if dt_name == "bfloat16" else 3)
    obufs = 4 if dt_name in ("bfloat16", "float8e3") else 2
    pbufs = 6 if _layout(C, dt_name)[1] == 1 else 2
    # y stores batched: big groups early (4KB per-partition lines), tapering
    # to singles at the end so the last store doesn't lengthen the tail
    if dt_name == "float8e3":
        ybatch = [4, 4, 4, 2, 1, 1]
    else:
        ybatch = [1] * EPC
    assert sum(ybatch) == EPC
    # w stream: one 1MB DMA per expert slab (16 in-flight small DMAs pack the
    # queues better than few large ones — measured), with the first and last
    # two slabs quartered per-I-chunk (earlier first matmul, shorter tail)
    SLAB = I_CHUNKS * H  # columns per expert slab
    nc = bacc.Bacc()
    # wc host-prearranged: [b, p, i*H + h] = w[b, i*128+p, h]; each expert's
    # slab is one dense 1MB DRAM block so a slab DMA's descriptors land in
    # one contiguous region (flat [P, EPC*SLAB] layout measured ~12µs slower:
    # 2MB-strided descriptors lose DRAM locality)
    wc = nc.declare_dram_parameter("wc", [EPC, P, SLAB], wdt, isOutput=False)
    xT = nc.declare_dram_parameter("xT", [I_DIM, EPC * C], xdt, isOutput=False)
    # Output packing: G = 128//C PE column groups; expert b's H chunk h goes
    # to psum partitions (h%G)*C..+C, bank cols (h//G)*512..+512, so stores
    # use all 128 partitions. Host unpacks.
    G, NB, PPART = _layout(C, dt_name)
    y = nc.declare_dram_parameter(
        "y", [PPART, EPC * NB * H_CHUNK], ydt, isOutput=True)

    with tile.TileContext(nc) as tc:
        with (
            tc.tile_pool(name="wp", bufs=6) as wp,
            tc.tile_pool(name="xp", bufs=1) as xp,
            tc.tile_pool(name="pp", bufs=pbufs, space="PSUM") as pp,
            tc.tile_pool(name="op", bufs=obufs) as op,
        ):
            # first expert slab issued before anything else, quartered across
            # both HWDGE rings so both sets of queues start streaming at once
            wt0 = wp.tile([P, SLAB], wdt, tag="w0", name="w0", bufs=wbufs)
            for i in range(I_CHUNKS):
                ring = nc.sync if i % 2 == 0 else nc.scalar
                ring.dma_start(out=wt0[:, i * H:(i + 1) * H],
                               in_=wc[0, :, i * H:(i + 1) * H])

            # x rows (stationary operands), resident for the whole kernel
            xtiles = []
            for i in range(I_CHUNKS):
                xt = xp.tile([P, EPC * C], xdt, tag=f"x{i}", name=f"x{i}")
                ring = nc.scalar if i % 2 == 0 else nc.sync
                ring.dma_start(out=xt[:], in_=xT[i * P:(i + 1) * P, :])
                xtiles.append(xt)

            # expert b -> (store group, offset of b's slot in the group tile)
            ygrp = []
            off = 0
            for gi, sz in enumerate(ybatch):
                for j in range(sz):
                    ygrp.append((gi, off, sz, j))
                off += sz
            OW = NB * H_CHUNK
            ot = None
            for b in range(EPC):
                if b == 0:
                    wt = wt0
                else:
                    wt = wp.tile([P, SLAB], wdt, tag="w0",
                                 name=f"w{b}", bufs=wbufs)
                    # w descriptor-gen alternates between the SP and ACT
                    # HWDGE rings so neither sequencer serializes the stream
                    ring = nc.sync if b % 2 == 0 else nc.scalar
                    if b >= EPC - 2:
                        # tail experts: per-I-chunk DMAs so matmuls start on
                        # partial slabs, shortening the post-stream tail
                        for i in range(I_CHUNKS):
                            ring.dma_start(out=wt[:, i * H:(i + 1) * H],
                                           in_=wc[b, :, i * H:(i + 1) * H])
                    else:
                        ring.dma_start(out=wt[:], in_=wc[b])
                ps = pp.tile([PPART, NB * H_CHUNK], mybir.dt.float32,
                             tag="ps", name=f"ps{b}")
                for i in range(I_CHUNKS):
                    for h in range(H_CHUNKS):
                        g, bank = h % G, h // G
                        nc.tensor.matmul(
                            ps[g * C:(g + 1) * C,
                               bank * H_CHUNK:(bank + 1) * H_CHUNK],
                            lhsT=xtiles[i][:, b * C:(b + 1) * C],
                            rhs=wt[:, i * H + h * H_CHUNK: i * H + (h + 1) * H_CHUNK],
                            start=(i == 0),
                            stop=(i == I_CHUNKS - 1),
                            tile_position=(0, g * C) if G > 1 else None,
                        )
                gi, goff, gsz, gj = ygrp[b]
                if gj == 0:
                    ot = op.tile([PPART, gsz * OW], ydt, tag="o",
                                 name=f"o{gi}", bufs=obufs)
                nc.vector.tensor_copy(out=ot[:, gj * OW:(gj + 1) * OW],
                                      in_=ps[:])
                # y store groups alternate rings to keep ring bytes balanced
                if gj == gsz - 1:
                    ring = nc.scalar if gi % 2 == 0 else nc.sync
                    ring.dma_start(
                        out=y[:, goff * OW:(goff + gsz) * OW], in_=ot[:])
    nc.compile()
    return nc


def _get_nc(C: int, dt_name: str):
    key = (C, dt_name)
    if key not in _cache:
        _cache[key] = _build(C, dt_name)
    return _cache[key]


def _prepare(x, w, chosen_experts, expert_weight, dt_name):
    """Host-side routing. Returns (in_maps, row_lists) where row_lists[c][s]
    is the array of global row ids for core c, expert slot s."""
    x = np.asarray(x, dtype=np.float32)
    w = np.asarray(w, dtype=np.float32)
    ce = np.asarray(chosen_experts).astype(np.int64).reshape(-1)      # [T*K]
    gw = np.asarray(expert_weight, dtype=np.float32).reshape(-1)      # [T*K]

    wdt_name, xdt_name, _ = _dtypes(dt_name)
    import ml_dtypes
    _np = {"float32": np.float32, "float32r": np.float32,
           "bfloat16": ml_dtypes.bfloat16, "float8e3": ml_dtypes.float8_e3m4}
    w_np, x_np = _np[wdt_name], _np[xdt_name]

    counts = np.bincount(ce, minlength=E)
    C = max(32, int(np.ceil(counts.max() / 32.0) * 32))

    order = np.argsort(ce, kind="stable")
    starts = np.zeros(E + 1, dtype=np.int64)
    np.cumsum(counts, out=starts[1:])

    xs = x * gw[:, None]  # fold router gate into rows (fp32)
    if wdt_name == "float8e3":
        # per-expert scale: map max|w_e| -> W8_TARGET so the gaussian bulk
        # sits in e3m4's normal range; the inverse folds into the x rows
        # (each row is used by exactly one expert), so the device kernel
        # needs no descaling.
        wmax = np.abs(w).max(axis=(1, 2))                 # [E]
        wscale = (W8_TARGET / np.maximum(wmax, 1e-30)).astype(np.float32)
        w = w * wscale[:, None, None]
        xs = xs / wscale[ce, None]

    in_maps, row_lists = [], []
    for c in range(N_CORES):
        xg = np.zeros((EPC * C, I_DIM), dtype=np.float32)
        rows_c = []
        for s in range(EPC):
            e = c * EPC + s
            rows = order[starts[e]:starts[e + 1]]
            xg[s * C: s * C + len(rows)] = xs[rows]
            rows_c.append(rows)
        # [b, i*128+p, h] -> [b, p, i*H + h]: contiguous per-partition slab
        # lines, one dense 1MB block per expert (see wc decl in _build)
        wcore = (
            w[c * EPC:(c + 1) * EPC]
            .reshape(EPC, I_CHUNKS, P, H)
            .transpose(0, 2, 1, 3)
            .reshape(EPC, P, I_CHUNKS * H)
        )
        in_maps.append({
            "wc": np.ascontiguousarray(wcore).astype(w_np),
            "xT": np.ascontiguousarray(xg.T).astype(x_np),
        })
        row_lists.append(rows_c)
    return C, in_maps, row_lists


def _combine(results, row_lists, C, dt_name):
    G, NB, PPART = _layout(C, dt_name)
    yfull = np.empty((T * K_TOP, H), dtype=np.float32)
    for c in range(N_CORES):
        yc = np.asarray(results[c]["y"], dtype=np.float32)
        # [G*C, EPC*NB*512]: partition (g*C+r), col (b*NB*512+bank*512+hc)
        # -> expert b, out[r, (bank*G+g)*512+hc]
        yc = (yc.reshape(G, C, EPC, NB, H_CHUNK)
                .transpose(2, 1, 3, 0, 4).reshape(EPC, C, H))
        for s, rows in enumerate(row_lists[c]):
            if len(rows):
                yfull[rows] = yc[s, : len(rows)]
    return yfull[0::2] + yfull[1::2]


def run(x, w, chosen_experts, expert_weight, dt_name=DEFAULT_DTYPE, **spmd_kwargs):
    from concourse.bass_utils import run_bass_kernel_spmd

    C, in_maps, row_lists = _prepare(x, w, chosen_experts, expert_weight, dt_name)
    nc = _get_nc(C, dt_name)
    res = run_bass_kernel_spmd(nc, in_maps, core_ids=list(range(N_CORES)), **spmd_kwargs)
    out = _combine(res.results, row_lists, C, dt_name)
    return out, res


def kernel(x, w, chosen_experts, expert_weight):
    out, _ = run(x, w, chosen_experts, expert_weight)
    return out

